# revision 16
# baseline (speedup 1.0000x reference)
"""Bahdanau attention (B=32, Lk=4096, H=512) on 8 Trainium2 NeuronCores.

Data-parallel over batch: core c handles batches [4c, 4c+4). No
collectives; each core computes its batches end to end.

Per-core device program (B_loc=4), per batch:
  qq       = Wa_w @ q + (Wa_b + Ua_b)                   (PE f32r, tiny)
  pre[:,l] = Ua_w @ keys[b,l]   (PSUM-accumulated)      (PE f32r, [h',l] layout)
  t        = tanh(pre + qq[b])                          (ACT, bias fused, bf16 out)
  partials = va_mt . t_mt  (4 col-packed matmuls,       (PE bf16, tile_position:
             concurrent in the 128x128 array)            rows 0/32/64/96 of PSUM)
  s        = sum(partials) + maskbias                   (PE f32r K=98 matmul over a
                                                         DVE-copied [98,512] tile)
  e, zp   += exp(s/T + Va_b/T)                          (ACT from PSUM, fused accum,
                                                         f32r out)
  wb       = ones x e-chunk (broadcast to 128 rows)     (PE f32r K=1 matmul)
  ctx_part+= sum_l e[l]*keysT[h,l]  (unnormalized)      (DVE scalar_tensor_tensor,
                                                         fused mult+accum)
  tail: z = sum(zp); w = e/z -> wout; ctx = ctx_part/z  (DVE + tiny DMAs; deferred
                                                         one batch for overlap)

Key structural points:
 - keys arrive host-pre-transposed as [B_loc, H, Lk] so the contraction
   dim h lies on SBUF partitions for the PE matmul; the ctx reduction
   then runs over the free dim on DVE. No on-chip transposes.
 - f32r (single-pass fp32, ~tf32 rounding) runs at full PE rate for
   moving dims >= 256; plain fp32 matmul would be 4x slower.
 - masking is an additive -30000*T score bias; exp underflows to exact
   0, matching the reference's -1e9 fill + softmax semantics.
 - softmax needs no max-subtraction: |s| <= sum|Va| + |Va_b| ~ 12.
 - ctx accumulates unnormalized per 512-chunk right after each exp (no
   wait on the full-row softmax denominator), so only a tiny tail
   remains per batch, software-pipelined against the next batch.
 - custom-DVE table ops (tensor_tensor_reduce etc.) crash this runtime;
   only native opcodes are used (scalar_tensor_tensor, tensor_reduce,
   tensor_scalar, reciprocal).
 - DVE/ACT row ops keep base partition 0 and chunks < 2048 where a
   nonzero base partition is involved (walrus ISA limits).

Aux inputs: qTr [128,17] f32r (query^T chunks + ones98 col for the
sum/broadcast matmuls), var32 [128,128] bf16 (Va chunks replicated x32
for col-packing), auxf [128,8] f32 (Wa_b+Ua_b chunks; [0,4]=Va_b/T;
[0,5]=1/T), maskb [B_loc,Lk] f32r, onesr [1,128] f32r.
"""

import numpy as np

B, LK, H = 32, 4096, 512
NCORES = 8
BLOC = B // NCORES  # 4
NLT = LK // 512     # 8 l-tiles of 512
NKC = H // 128      # 4 contraction chunks
NMT = H // 128      # 4 output-feature chunks
CTX_CHUNK = 2048    # free-dim chunk for the ctx fused mult+sum
NCC = LK // CTX_CHUNK

_CACHE = {}


def _build_program():
    import concourse.bacc as bacc
    import concourse.mybir as mybir
    from concourse import tile

    f32 = mybir.dt.float32
    f32r = mybir.dt.float32r
    bf16 = mybir.dt.bfloat16
    AF = mybir.ActivationFunctionType
    ALU = mybir.AluOpType

    nc = bacc.Bacc(None, target_bir_lowering=False, debug=False)

    keysT = nc.dram_tensor("keysT", [BLOC, H, LK], f32r, kind="ExternalInput")
    uaT = nc.dram_tensor("uaT", [H, H], f32r, kind="ExternalInput")
    waT = nc.dram_tensor("waT", [H, H], f32r, kind="ExternalInput")
    qTr = nc.dram_tensor("qTr", [128, NKC * BLOC + 1], f32r, kind="ExternalInput")
    var32 = nc.dram_tensor("var32", [128, NMT * 32], bf16, kind="ExternalInput")
    auxf = nc.dram_tensor("auxf", [128, 8], f32, kind="ExternalInput")
    maskb = nc.dram_tensor("maskb", [BLOC, LK], f32r, kind="ExternalInput")
    onesr = nc.dram_tensor("onesr", [1, 128], f32r, kind="ExternalInput")

    wout = nc.dram_tensor("wout", [BLOC, LK], f32, kind="ExternalOutput")
    ctxout = nc.dram_tensor("ctxout", [BLOC, H], f32, kind="ExternalOutput")

    with tile.TileContext(nc) as tc:
        with (
            tc.tile_pool(name="const", bufs=1) as cpool,
            tc.tile_pool(name="vs", bufs=2, space="PSUM") as vspool,
        ):
            # ---- constants to SBUF ----
            uaT_sb = []
            for kc in range(NKC):
                t = cpool.tile([128, H], f32r, tag=f"uaT{kc}")
                nc.sync.dma_start(out=t[:], in_=uaT[kc * 128:(kc + 1) * 128, :])
                uaT_sb.append(t)
            qTr_sb = cpool.tile([128, NKC * BLOC + 1], f32r, tag="qTr")
            nc.sync.dma_start(out=qTr_sb[:], in_=qTr[:])
            var32_sb = cpool.tile([128, NMT * 32], bf16, tag="var32")
            nc.sync.dma_start(out=var32_sb[:], in_=var32[:])
            auxf_sb = cpool.tile([128, 8], f32, tag="auxf")
            nc.sync.dma_start(out=auxf_sb[:], in_=auxf[:])
            onesr_sb = cpool.tile([1, 128], f32r, tag="onesr")
            nc.sync.dma_start(out=onesr_sb[:], in_=onesr[:])

            ones98_c = qTr_sb[0:98, NKC * BLOC:NKC * BLOC + 1]
            bsum_c = lambda mt: auxf_sb[:, mt:mt + 1]
            vabT_c = auxf_sb[0:1, 4:5]
            invT_c = auxf_sb[0:1, 5:6]

            # qq[:, mt*BLOC + b] = (Wa q[b] + Wa_b + Ua_b) chunk mt
            qq_sb = cpool.tile([128, NMT * BLOC], f32, tag="qq")

            # ---- q projection (waT pool freed before the big pools) ----
            with tc.tile_pool(name="wa", bufs=1) as wpool:
                waT_sb = []
                for kc in range(NKC):
                    t = wpool.tile([128, H], f32r, tag=f"waT{kc}")
                    nc.sync.dma_start(out=t[:], in_=waT[kc * 128:(kc + 1) * 128, :])
                    waT_sb.append(t)
                for mt in range(NMT):
                    ps = vspool.tile([128, BLOC], f32, tag="vs")
                    for kc in range(NKC):
                        nc.tensor.matmul(
                            ps[:],
                            lhsT=waT_sb[kc][:, mt * 128:(mt + 1) * 128],
                            rhs=qTr_sb[:, kc * BLOC:(kc + 1) * BLOC],
                            start=(kc == 0),
                            stop=(kc == NKC - 1),
                        )
                    nc.vector.tensor_scalar(
                        out=qq_sb[:, mt * BLOC:(mt + 1) * BLOC], in0=ps[:],
                        scalar1=bsum_c(mt), scalar2=None, op0=ALU.add,
                    )

            with (
                tc.tile_pool(name="keys", bufs=2 * NKC * NCC) as kpool,
                tc.tile_pool(name="tanh", bufs=6) as tpool,
                tc.tile_pool(name="scr", bufs=2) as spool,
                tc.tile_pool(name="mbp", bufs=3) as mbpool,
                tc.tile_pool(name="rows", bufs=2) as rpool,
                tc.tile_pool(name="acc", bufs=2) as apool,
                tc.tile_pool(name="mm", bufs=3, space="PSUM") as mmpool,
                tc.tile_pool(name="psv", bufs=2, space="PSUM") as psvpool,
                tc.tile_pool(name="wbp", bufs=1, space="PSUM") as wbppool,
                tc.tile_pool(name="dram", bufs=2, space="DRAM") as dpool,
            ):
                def compute_phase(b):
                    e_row = rpool.tile([1, LK], f32r, tag="e_row", name="e_row")
                    zp = apool.tile([1, NLT], f32, tag="zp", name="zp")
                    parts = [apool.tile([128, NLT], f32, tag=f"part{hc}",
                                        name=f"part{hc}")
                             for hc in range(NKC)]
                    kts = {}
                    for cc in range(NCC):
                        for kc in range(NKC):
                            kt = kpool.tile([128, CTX_CHUNK], f32r, tag="kt",
                                            name="kt")
                            nc.sync.dma_start(
                                out=kt[:],
                                in_=keysT[b, kc * 128:(kc + 1) * 128,
                                          cc * CTX_CHUNK:(cc + 1) * CTX_CHUNK],
                            )
                            kts[(kc, cc)] = kt
                    for lt in range(NLT):
                        l0 = lt * 512
                        ths = []
                        for mt in range(NMT):
                            ps = mmpool.tile([128, 512], f32, tag="mm",
                                             name="ps")
                            cc, co = lt * 512 // CTX_CHUNK, (lt * 512) % CTX_CHUNK
                            for kc in range(NKC):
                                nc.tensor.matmul(
                                    ps[:],
                                    lhsT=uaT_sb[kc][:, mt * 128:(mt + 1) * 128],
                                    rhs=kts[(kc, cc)][:, co:co + 512],
                                    start=(kc == 0),
                                    stop=(kc == NKC - 1),
                                )
                            th = tpool.tile([128, 512], bf16, tag="th",
                                            name="th")
                            nc.scalar.activation(
                                out=th[:], in_=ps[:], func=AF.Tanh,
                                bias=qq_sb[:, mt * BLOC + b:mt * BLOC + b + 1],
                                scale=1.0,
                            )
                            ths.append(th)
                        # scores: 4 col-packed (32-replicated) Va matmuls
                        # run concurrently in the PE array; partials land at
                        # psum rows {0,32,64,96}; one K=98 f32r matmul then
                        # sums them + the mask row
                        psv = psvpool.tile([128, 512], f32, tag="psv",
                                           name="psv")
                        for j in range(NMT):
                            nc.tensor.matmul(
                                psv[32 * j:32 * j + 32, :],
                                lhsT=var32_sb[:, 32 * j:32 * j + 32],
                                rhs=ths[j][:],
                                start=True, stop=True,
                                tile_position=(0, 32 * j),
                            )
                        vt98 = mbpool.tile([98, 512], f32r, tag="vt98",
                                           name="vt98")
                        nc.sync.dma_start(out=vt98[97:98, :],
                                          in_=maskb[b:b + 1, l0:l0 + 512])
                        nc.vector.tensor_copy(vt98[0:97, :], psv[0:97, :])
                        ss = vspool.tile([1, 512], f32, tag="vs", name="ss")
                        nc.tensor.matmul(ss[:], lhsT=ones98_c, rhs=vt98[:],
                                         start=True, stop=True)
                        # e = exp(s/T + Va_b/T) in f32r, fused sum into zp
                        nc.scalar.activation(
                            out=e_row[0:1, l0:l0 + 512], in_=ss[:], func=AF.Exp,
                            bias=vabT_c, scale=invT_c,
                            accum_out=zp[0:1, lt:lt + 1],
                        )
                        # broadcast e chunk to 128 partitions on PE (ones x e)
                        wb = wbppool.tile([128, 512], f32, tag="wbp",
                                          name="wb")
                        nc.tensor.matmul(
                            wb[:], lhsT=onesr_sb[:],
                            rhs=e_row[0:1, l0:l0 + 512],
                            start=True, stop=True,
                        )
                        # unnormalized ctx accumulation: parts[hc][:,lt] =
                        # sum_l e[l]*keysT[h,l] over this chunk
                        for hc in range(NKC):
                            prod = spool.tile([128, 512], f32, tag="prod",
                                              name="prod")
                            nc.vector.scalar_tensor_tensor(
                                out=prod[:],
                                in0=kts[(hc, cc)][:, co:co + 512].bitcast(f32),
                                scalar=1.0, in1=wb[:],
                                op0=ALU.mult, op1=ALU.mult,
                                accum_out=parts[hc][:, lt:lt + 1],
                            )
                    return e_row, zp, parts

                def tail_phase(b, e_row, zp, parts):
                    # normalize (no max-sub needed: |s| <= sum|Va| ~ 12)
                    z = apool.tile([1, 1], f32, tag="z", name="z")
                    nc.vector.tensor_reduce(out=z[:], in_=zp[:],
                                            axis=mybir.AxisListType.X, op=ALU.add)
                    rz = apool.tile([1, 1], f32, tag="rz", name="rz")
                    nc.vector.reciprocal(rz[:], z[:])
                    nc.vector.tensor_scalar(out=e_row[:], in0=e_row[:],
                                            scalar1=rz[:], scalar2=None,
                                            op0=ALU.mult)
                    nc.sync.dma_start(out=wout[b:b + 1, :],
                                      in_=e_row[:].bitcast(f32))
                    # broadcast 1/z to 128 partitions via a DRAM bounce
                    drz = dpool.tile([1, 1], f32, tag="drz", name="drz")
                    nc.sync.dma_start(out=drz[:], in_=rz[:])
                    rzb = apool.tile([128, 1], f32, tag="rzb", name="rzb")
                    nc.sync.dma_start(out=rzb[:],
                                      in_=drz[0:1, 0:1].to_broadcast((128, 1)))
                    for hc in range(NKC):
                        raw = apool.tile([128, 1], f32, tag="raw", name="raw")
                        nc.vector.tensor_reduce(out=raw[:], in_=parts[hc][:],
                                                axis=mybir.AxisListType.X,
                                                op=ALU.add)
                        ctxv = apool.tile([128, 1], f32, tag="ctxv",
                                          name="ctxv")
                        nc.vector.tensor_scalar(out=ctxv[:], in0=raw[:],
                                                scalar1=rzb[:], scalar2=None,
                                                op0=ALU.mult)
                        out_ap = ctxout[b, hc * 128:(hc + 1) * 128].rearrange(
                            "(h x) -> h x", x=1
                        )
                        nc.sync.dma_start(out=out_ap, in_=ctxv[:])

                # software-pipelined: batch b's softmax+ctx tail runs during
                # batch b+1's matmul phase, so no DMA/engine wait blocks the
                # next batch's key loads or matmuls behind it in FIFO order
                pending = None
                for b in range(BLOC):
                    state = compute_phase(b)
                    if pending is not None:
                        tail_phase(b - 1, *pending)
                    pending = state
                tail_phase(BLOC - 1, *pending)

    nc.compile()
    return nc


def _get_program():
    if "nc" not in _CACHE:
        _CACHE["nc"] = _build_program()
    return _CACHE["nc"]


def _pack_small(query_loc, Va_w, Wa_b, Ua_b, Va_b, T):
    import ml_dtypes
    qTr = np.zeros((128, NKC * BLOC + 1), np.float32)
    for r in (0, 32, 64, 96, 97):
        qTr[r, NKC * BLOC] = 1.0
    for kc in range(NKC):
        qTr[:, kc * BLOC:(kc + 1) * BLOC] = \
            query_loc[:, kc * 128:(kc + 1) * 128].T
    var32 = np.repeat(Va_w.reshape(-1).reshape(NMT, 128).T, 32,
                      axis=1).astype(ml_dtypes.bfloat16)
    auxf = np.zeros((128, 8), np.float32)
    auxf[:, 0:4] = (Wa_b + Ua_b).reshape(NMT, 128).T
    auxf[0, 4] = float(Va_b.reshape(-1)[0]) / T
    auxf[0, 5] = 1.0 / T
    return qTr, var32, auxf


def kernel(query, keys, Wa_w, Wa_b, Ua_w, Ua_b, Va_w, Va_b, temperature,
           valid_src_len):
    import ml_dtypes
    from concourse.bass_utils import run_bass_kernel_spmd

    query = np.asarray(query, dtype=np.float32)
    keys = np.asarray(keys, dtype=np.float32)
    Wa_w = np.asarray(Wa_w, dtype=np.float32)
    Wa_b = np.asarray(Wa_b, dtype=np.float32)
    Ua_w = np.asarray(Ua_w, dtype=np.float32)
    Ua_b = np.asarray(Ua_b, dtype=np.float32)
    Va_w = np.asarray(Va_w, dtype=np.float32)
    Va_b = np.asarray(Va_b, dtype=np.float32)
    temperature = np.asarray(temperature, dtype=np.float32)
    vlen = np.asarray(valid_src_len).astype(np.int64)

    T = float(temperature.reshape(-1)[0])
    uaT = np.ascontiguousarray(Ua_w.T)
    waT = np.ascontiguousarray(Wa_w.T)
    valid = np.arange(LK)[None, :] < vlen[:, None]
    maskb_full = np.where(valid, 0.0, -30000.0 * max(abs(T), 1.0)).astype(
        np.float32)
    keysT_full = np.ascontiguousarray(np.swapaxes(keys, 1, 2))  # [B, H, LK]

    in_maps = []
    for c in range(NCORES):
        s = slice(c * BLOC, (c + 1) * BLOC)
        qTr, var32, auxf = _pack_small(query[s, 0, :], Va_w, Wa_b,
                                       Ua_b, Va_b, T)
        in_maps.append({
            "keysT": keysT_full[s],
            "uaT": uaT,
            "waT": waT,
            "qTr": qTr,
            "var32": var32,
            "auxf": auxf,
            "maskb": np.ascontiguousarray(maskb_full[s]),
            "onesr": np.ones((1, 128), np.float32),
        })

    nc = _get_program()
    res = run_bass_kernel_spmd(nc, in_maps, list(range(NCORES)))

    context = np.empty((B, 1, H), np.float32)
    weights = np.empty((B, 1, LK), np.float32)
    for c in range(NCORES):
        r = res.results[c]
        context[c * BLOC:(c + 1) * BLOC, 0, :] = r["ctxout"]
        weights[c * BLOC:(c + 1) * BLOC, 0, :] = r["wout"]
    return context, weights


# revision 26
# speedup vs baseline: 1.0053x; 1.0053x over previous
"""Bahdanau attention (B=32, Lk=4096, H=512) on 8 Trainium2 NeuronCores.

Data-parallel over batch: core c handles batches [4c, 4c+4). No
collectives; each core computes its batches end to end.

Per-core device program (B_loc=4), per batch:
  qq       = Wa_w @ q + (Wa_b + Ua_b)                   (PE f32r, tiny)
  pre[:,l] = Ua_w @ keys[b,l]   (PSUM-accumulated)      (PE f32r, [h',l] layout)
  t        = tanh(pre + qq[b])                          (ACT, bias fused, bf16 out)
  partials = va_mt . t_mt  (4 col-packed matmuls,       (PE bf16, tile_position:
             concurrent in the 128x128 array)            rows 0/32/64/96 of PSUM)
  s        = sum(partials) + maskbias                   (PE f32r K=98 matmul over a
                                                         DVE-copied [98,512] tile)
  e, zp   += exp(s/T + Va_b/T)                          (ACT from PSUM, fused accum,
                                                         f32r out)
  wb       = ones x e-chunk (broadcast to 128 rows)     (PE f32r K=1 matmul)
  ctx_part+= sum_l e[l]*keysT[h,l]  (unnormalized)      (DVE scalar_tensor_tensor,
                                                         fused mult+accum)
  tail: z = sum(zp); w = e/z -> wout; ctx = ctx_part/z  (DVE + tiny DMAs; deferred
                                                         one batch for overlap)

Key structural points:
 - keys arrive host-pre-transposed as [B_loc, H, Lk] so the contraction
   dim h lies on SBUF partitions for the PE matmul; the ctx reduction
   then runs over the free dim on DVE. No on-chip transposes.
 - f32r (single-pass fp32, ~tf32 rounding) runs at full PE rate for
   moving dims >= 256; plain fp32 matmul would be 4x slower.
 - masking is an additive -30000*T score bias; exp underflows to exact
   0, matching the reference's -1e9 fill + softmax semantics.
 - softmax needs no max-subtraction: |s| <= sum|Va| + |Va_b| ~ 12.
 - ctx accumulates unnormalized per 512-chunk right after each exp (no
   wait on the full-row softmax denominator), so only a tiny tail
   remains per batch, software-pipelined against the next batch.
 - custom-DVE table ops (tensor_tensor_reduce etc.) crash this runtime;
   only native opcodes are used (scalar_tensor_tensor, tensor_reduce,
   tensor_scalar, reciprocal).
 - DVE/ACT row ops keep base partition 0 and chunks < 2048 where a
   nonzero base partition is involved (walrus ISA limits).

Aux inputs: qTr [128,17] f32r (query^T chunks + ones98 col for the
sum/broadcast matmuls), var32 [128,128] bf16 (Va chunks replicated x32
for col-packing), auxf [128,8] f32 (Wa_b+Ua_b chunks; [0,4]=Va_b/T;
[0,5]=1/T), maskb [B_loc,Lk] f32r, onesr [1,128] f32r.
"""

import numpy as np

B, LK, H = 32, 4096, 512
NCORES = 8
BLOC = B // NCORES  # 4
NLT = LK // 512     # 8 l-tiles of 512
NKC = H // 128      # 4 contraction chunks
NMT = H // 128      # 4 output-feature chunks
CTX_CHUNK = 2048    # free-dim chunk for the ctx fused mult+sum
NCC = LK // CTX_CHUNK

_CACHE = {}


def _build_program():
    import concourse.bacc as bacc
    import concourse.mybir as mybir
    from concourse import tile

    f32 = mybir.dt.float32
    f32r = mybir.dt.float32r
    bf16 = mybir.dt.bfloat16
    AF = mybir.ActivationFunctionType
    ALU = mybir.AluOpType

    nc = bacc.Bacc(None, target_bir_lowering=False, debug=False)

    keysT = nc.dram_tensor("keysT", [BLOC, H, LK], f32r, kind="ExternalInput")
    uaT = nc.dram_tensor("uaT", [H, H], f32r, kind="ExternalInput")
    waT = nc.dram_tensor("waT", [H, H], f32r, kind="ExternalInput")
    qTr = nc.dram_tensor("qTr", [128, NKC * BLOC + 1], f32r, kind="ExternalInput")
    var32 = nc.dram_tensor("var32", [128, NMT * 32], bf16, kind="ExternalInput")
    auxf = nc.dram_tensor("auxf", [128, 8], f32, kind="ExternalInput")
    maskb = nc.dram_tensor("maskb", [BLOC, LK], f32r, kind="ExternalInput")
    onesr = nc.dram_tensor("onesr", [1, 128], f32r, kind="ExternalInput")

    wout = nc.dram_tensor("wout", [BLOC, LK], f32, kind="ExternalOutput")
    ctxout = nc.dram_tensor("ctxout", [BLOC, H], f32, kind="ExternalOutput")

    with tile.TileContext(nc) as tc:
        with (
            tc.tile_pool(name="const", bufs=1) as cpool,
            tc.tile_pool(name="vs", bufs=2, space="PSUM") as vspool,
        ):
            # ---- constants to SBUF (small tiles first: the earliest PE
            # work, the q-projection, needs qTr/auxf; uaT can trail) ----
            qTr_sb = cpool.tile([128, NKC * BLOC + 1], f32r, tag="qTr")
            nc.sync.dma_start(out=qTr_sb[:], in_=qTr[:])
            var32_sb = cpool.tile([128, NMT * 32], bf16, tag="var32")
            nc.sync.dma_start(out=var32_sb[:], in_=var32[:])
            auxf_sb = cpool.tile([128, 8], f32, tag="auxf")
            nc.sync.dma_start(out=auxf_sb[:], in_=auxf[:])
            onesr_sb = cpool.tile([1, 128], f32r, tag="onesr")
            nc.sync.dma_start(out=onesr_sb[:], in_=onesr[:])
            uaT_sb = []
            uaT_tiles = [cpool.tile([128, H], f32r, tag=f"uaT{kc}",
                                    name=f"uaT{kc}")
                         for kc in range(NKC)]

            ones98_c = qTr_sb[0:98, NKC * BLOC:NKC * BLOC + 1]
            bsum_c = lambda mt: auxf_sb[:, mt:mt + 1]
            vabT_c = auxf_sb[0:1, 4:5]
            invT_c = auxf_sb[0:1, 5:6]

            # qq[:, mt*BLOC + b] = (Wa q[b] + Wa_b + Ua_b) chunk mt
            qq_sb = cpool.tile([128, NMT * BLOC], f32, tag="qq")

            # ---- q projection (waT pool freed before the big pools) ----
            with tc.tile_pool(name="wa", bufs=1) as wpool:
                waT_sb = []
                for kc in range(NKC):
                    t = wpool.tile([128, H], f32r, tag=f"waT{kc}")
                    nc.sync.dma_start(out=t[:], in_=waT[kc * 128:(kc + 1) * 128, :])
                    waT_sb.append(t)
                for kc in range(NKC):
                    nc.sync.dma_start(out=uaT_tiles[kc][:],
                                      in_=uaT[kc * 128:(kc + 1) * 128, :])
                    uaT_sb.append(uaT_tiles[kc])
                for mt in range(NMT):
                    ps = vspool.tile([128, BLOC], f32, tag="vs")
                    for kc in range(NKC):
                        nc.tensor.matmul(
                            ps[:],
                            lhsT=waT_sb[kc][:, mt * 128:(mt + 1) * 128],
                            rhs=qTr_sb[:, kc * BLOC:(kc + 1) * BLOC],
                            start=(kc == 0),
                            stop=(kc == NKC - 1),
                        )
                    nc.vector.tensor_scalar(
                        out=qq_sb[:, mt * BLOC:(mt + 1) * BLOC], in0=ps[:],
                        scalar1=bsum_c(mt), scalar2=None, op0=ALU.add,
                    )

            with (
                tc.tile_pool(name="keys", bufs=2 * NKC * NCC) as kpool,
                tc.tile_pool(name="tanh", bufs=6) as tpool,
                tc.tile_pool(name="scr", bufs=2) as spool,
                tc.tile_pool(name="mbp", bufs=3) as mbpool,
                tc.tile_pool(name="rows", bufs=2) as rpool,
                tc.tile_pool(name="acc", bufs=2) as apool,
                tc.tile_pool(name="mm", bufs=3, space="PSUM") as mmpool,
                tc.tile_pool(name="psv", bufs=2, space="PSUM") as psvpool,
                tc.tile_pool(name="wbp", bufs=1, space="PSUM") as wbppool,
                tc.tile_pool(name="dram", bufs=2, space="DRAM") as dpool,
            ):
                def compute_phase(b):
                    e_row = rpool.tile([1, LK], f32r, tag="e_row", name="e_row")
                    zp = apool.tile([1, NLT], f32, tag="zp", name="zp")
                    parts = [apool.tile([128, NLT], f32, tag=f"part{hc}",
                                        name=f"part{hc}")
                             for hc in range(NKC)]
                    kts = {}
                    for cc in range(NCC):
                        for kc in range(NKC):
                            kt = kpool.tile([128, CTX_CHUNK], f32r, tag="kt",
                                            name="kt")
                            nc.sync.dma_start(
                                out=kt[:],
                                in_=keysT[b, kc * 128:(kc + 1) * 128,
                                          cc * CTX_CHUNK:(cc + 1) * CTX_CHUNK],
                            )
                            kts[(kc, cc)] = kt
                    for lt in range(NLT):
                        l0 = lt * 512
                        ths = []
                        for mt in range(NMT):
                            ps = mmpool.tile([128, 512], f32, tag="mm",
                                             name="ps")
                            cc, co = lt * 512 // CTX_CHUNK, (lt * 512) % CTX_CHUNK
                            for kc in range(NKC):
                                nc.tensor.matmul(
                                    ps[:],
                                    lhsT=uaT_sb[kc][:, mt * 128:(mt + 1) * 128],
                                    rhs=kts[(kc, cc)][:, co:co + 512],
                                    start=(kc == 0),
                                    stop=(kc == NKC - 1),
                                )
                            th = tpool.tile([128, 512], bf16, tag="th",
                                            name="th")
                            nc.scalar.activation(
                                out=th[:], in_=ps[:], func=AF.Tanh,
                                bias=qq_sb[:, mt * BLOC + b:mt * BLOC + b + 1],
                                scale=1.0,
                            )
                            ths.append(th)
                        # scores: 4 col-packed (32-replicated) Va matmuls
                        # run concurrently in the PE array; partials land at
                        # psum rows {0,32,64,96}; one K=98 f32r matmul then
                        # sums them + the mask row
                        psv = psvpool.tile([128, 512], f32, tag="psv",
                                           name="psv")
                        for j in range(NMT):
                            nc.tensor.matmul(
                                psv[32 * j:32 * j + 32, :],
                                lhsT=var32_sb[:, 32 * j:32 * j + 32],
                                rhs=ths[j][:],
                                start=True, stop=True,
                                tile_position=(0, 32 * j),
                            )
                        vt98 = mbpool.tile([98, 512], f32r, tag="vt98",
                                           name="vt98")
                        nc.sync.dma_start(out=vt98[97:98, :],
                                          in_=maskb[b:b + 1, l0:l0 + 512])
                        nc.vector.tensor_copy(vt98[0:97, :], psv[0:97, :])
                        ss = vspool.tile([1, 512], f32, tag="vs", name="ss")
                        nc.tensor.matmul(ss[:], lhsT=ones98_c, rhs=vt98[:],
                                         start=True, stop=True)
                        # e = exp(s/T + Va_b/T) in f32r, fused sum into zp
                        nc.scalar.activation(
                            out=e_row[0:1, l0:l0 + 512], in_=ss[:], func=AF.Exp,
                            bias=vabT_c, scale=invT_c,
                            accum_out=zp[0:1, lt:lt + 1],
                        )
                        # broadcast e chunk to 128 partitions on PE (ones x e)
                        wb = wbppool.tile([128, 512], f32, tag="wbp",
                                          name="wb")
                        nc.tensor.matmul(
                            wb[:], lhsT=onesr_sb[:],
                            rhs=e_row[0:1, l0:l0 + 512],
                            start=True, stop=True,
                        )
                        # unnormalized ctx accumulation: parts[hc][:,lt] =
                        # sum_l e[l]*keysT[h,l] over this chunk
                        for hc in range(NKC):
                            prod = spool.tile([128, 512], f32, tag="prod",
                                              name="prod")
                            nc.vector.scalar_tensor_tensor(
                                out=prod[:],
                                in0=kts[(hc, cc)][:, co:co + 512].bitcast(f32),
                                scalar=1.0, in1=wb[:],
                                op0=ALU.mult, op1=ALU.mult,
                                accum_out=parts[hc][:, lt:lt + 1],
                            )
                    return e_row, zp, parts

                def tail_phase(b, e_row, zp, parts):
                    # normalize (no max-sub needed: |s| <= sum|Va| ~ 12)
                    z = apool.tile([1, 1], f32, tag="z", name="z")
                    nc.vector.tensor_reduce(out=z[:], in_=zp[:],
                                            axis=mybir.AxisListType.X, op=ALU.add)
                    rz = apool.tile([1, 1], f32, tag="rz", name="rz")
                    nc.vector.reciprocal(rz[:], z[:])
                    nc.vector.tensor_scalar(out=e_row[:], in0=e_row[:],
                                            scalar1=rz[:], scalar2=None,
                                            op0=ALU.mult)
                    nc.sync.dma_start(out=wout[b:b + 1, :],
                                      in_=e_row[:].bitcast(f32))
                    # broadcast 1/z to 128 partitions via a DRAM bounce
                    drz = dpool.tile([1, 1], f32, tag="drz", name="drz")
                    nc.sync.dma_start(out=drz[:], in_=rz[:])
                    rzb = apool.tile([128, 1], f32, tag="rzb", name="rzb")
                    nc.sync.dma_start(out=rzb[:],
                                      in_=drz[0:1, 0:1].to_broadcast((128, 1)))
                    for hc in range(NKC):
                        raw = apool.tile([128, 1], f32, tag="raw", name="raw")
                        nc.vector.tensor_reduce(out=raw[:], in_=parts[hc][:],
                                                axis=mybir.AxisListType.X,
                                                op=ALU.add)
                        ctxv = apool.tile([128, 1], f32, tag="ctxv",
                                          name="ctxv")
                        nc.vector.tensor_scalar(out=ctxv[:], in0=raw[:],
                                                scalar1=rzb[:], scalar2=None,
                                                op0=ALU.mult)
                        out_ap = ctxout[b, hc * 128:(hc + 1) * 128].rearrange(
                            "(h x) -> h x", x=1
                        )
                        nc.sync.dma_start(out=out_ap, in_=ctxv[:])

                # software-pipelined: batch b's softmax+ctx tail runs during
                # batch b+1's matmul phase, so no DMA/engine wait blocks the
                # next batch's key loads or matmuls behind it in FIFO order
                pending = None
                for b in range(BLOC):
                    state = compute_phase(b)
                    if pending is not None:
                        tail_phase(b - 1, *pending)
                    pending = state
                tail_phase(BLOC - 1, *pending)

    nc.compile()
    return nc


def _get_program():
    if "nc" not in _CACHE:
        _CACHE["nc"] = _build_program()
    return _CACHE["nc"]


def _pack_small(query_loc, Va_w, Wa_b, Ua_b, Va_b, T):
    import ml_dtypes
    qTr = np.zeros((128, NKC * BLOC + 1), np.float32)
    for r in (0, 32, 64, 96, 97):
        qTr[r, NKC * BLOC] = 1.0
    for kc in range(NKC):
        qTr[:, kc * BLOC:(kc + 1) * BLOC] = \
            query_loc[:, kc * 128:(kc + 1) * 128].T
    var32 = np.repeat(Va_w.reshape(-1).reshape(NMT, 128).T, 32,
                      axis=1).astype(ml_dtypes.bfloat16)
    auxf = np.zeros((128, 8), np.float32)
    auxf[:, 0:4] = (Wa_b + Ua_b).reshape(NMT, 128).T
    auxf[0, 4] = float(Va_b.reshape(-1)[0]) / T
    auxf[0, 5] = 1.0 / T
    return qTr, var32, auxf


def kernel(query, keys, Wa_w, Wa_b, Ua_w, Ua_b, Va_w, Va_b, temperature,
           valid_src_len):
    import ml_dtypes
    from concourse.bass_utils import run_bass_kernel_spmd

    query = np.asarray(query, dtype=np.float32)
    keys = np.asarray(keys, dtype=np.float32)
    Wa_w = np.asarray(Wa_w, dtype=np.float32)
    Wa_b = np.asarray(Wa_b, dtype=np.float32)
    Ua_w = np.asarray(Ua_w, dtype=np.float32)
    Ua_b = np.asarray(Ua_b, dtype=np.float32)
    Va_w = np.asarray(Va_w, dtype=np.float32)
    Va_b = np.asarray(Va_b, dtype=np.float32)
    temperature = np.asarray(temperature, dtype=np.float32)
    vlen = np.asarray(valid_src_len).astype(np.int64)

    T = float(temperature.reshape(-1)[0])
    uaT = np.ascontiguousarray(Ua_w.T)
    waT = np.ascontiguousarray(Wa_w.T)
    valid = np.arange(LK)[None, :] < vlen[:, None]
    maskb_full = np.where(valid, 0.0, -30000.0 * max(abs(T), 1.0)).astype(
        np.float32)
    keysT_full = np.ascontiguousarray(np.swapaxes(keys, 1, 2))  # [B, H, LK]

    in_maps = []
    for c in range(NCORES):
        s = slice(c * BLOC, (c + 1) * BLOC)
        qTr, var32, auxf = _pack_small(query[s, 0, :], Va_w, Wa_b,
                                       Ua_b, Va_b, T)
        in_maps.append({
            "keysT": keysT_full[s],
            "uaT": uaT,
            "waT": waT,
            "qTr": qTr,
            "var32": var32,
            "auxf": auxf,
            "maskb": np.ascontiguousarray(maskb_full[s]),
            "onesr": np.ones((1, 128), np.float32),
        })

    nc = _get_program()
    res = run_bass_kernel_spmd(nc, in_maps, list(range(NCORES)))

    context = np.empty((B, 1, H), np.float32)
    weights = np.empty((B, 1, LK), np.float32)
    for c in range(NCORES):
        r = res.results[c]
        context[c * BLOC:(c + 1) * BLOC, 0, :] = r["ctxout"]
        weights[c * BLOC:(c + 1) * BLOC, 0, :] = r["wout"]
    return context, weights
